# revision 1
# baseline (speedup 1.0000x reference)
"""ConvTransE forward on 8 Trainium2 NeuronCores (Bass/Tile).

Math shortcut: the reference computes scores = x @ ent.T  ([B, 100000]) and
returns scores[i, t[i]]; we only compute out[b] = x[b] . ent[t[b]].
The conv's retained slice [:, :512] depends only on ent[h] and rel[r][:, 0].

Sharding: tensor-parallel over the projection contraction dim (channels).
Core m owns channels [4m, 4m+4).  Every core:
  - gathers ent[h], ent[t], rel[r] rows on-device via indirect DMA,
  - PE-transposes overlapping 128-wide windows of the gathered rows,
  - runs the conv as banded matmuls on the PE (bands built on host from conv_w),
  - projects its K-slice:  z_m = relu(conv)_m @ proj_w_m^T   (K = 2048/core),
  - emits partial[b] = z_m[b] . ent[t[b]]  via a fused multiply+row-sum.
proj_b rides along as an extra "ones" contraction row on core 0 only.
Host sums the 8 [2048] partials.
"""

import numpy as np

NE, NRR, D, C, B = 100000, 500, 512, 32, 2048
NCORES = 8
CPC = C // NCORES          # 4 channels per core
NQ = B // 128              # 16 batch tiles of 128
CHUNK_BT = 2               # batch tiles per pipeline chunk
NCHUNK = NQ // CHUNK_BT
NB = CHUNK_BT * 128        # 256 batch columns per chunk
JB = 126                   # conv j-block (126 outputs need a 128-wide input window)
NSEG = 16                  # (c, s) main contraction blocks per core
KSTUB = CPC * 8 + 1        # 33: packed j=504..511 stub rows + ones row

_CACHE = {}


def _build_nc(mode="full"):
    from contextlib import ExitStack

    import concourse.bass as bass
    import concourse.tile as tile
    from concourse import bacc, mybir
    from concourse.masks import make_identity

    f32 = mybir.dt.float32
    i32 = mybir.dt.int32
    Alu = mybir.AluOpType

    nc = bacc.Bacc("TRN2", target_bir_lowering=False, debug=False,
                   num_devices=NCORES)

    ent = nc.dram_tensor("ent", [NE, D], f32, kind="ExternalInput")
    rel = nc.dram_tensor("rel", [NRR, D], f32, kind="ExternalInput")
    hI = nc.dram_tensor("hI", [128, NQ], i32, kind="ExternalInput")
    tI = nc.dram_tensor("tI", [128, NQ], i32, kind="ExternalInput")
    rI = nc.dram_tensor("rI", [128, NQ], i32, kind="ExternalInput")
    band = nc.dram_tensor("band", [128, NSEG * JB], f32, kind="ExternalInput")
    bstub = nc.dram_tensor("bstub", [10, 32], f32, kind="ExternalInput")
    pwt = nc.dram_tensor("pwt", [JB, NSEG * D], f32, kind="ExternalInput")
    pstub = nc.dram_tensor("pstub", [KSTUB, D], f32, kind="ExternalInput")
    cbias = nc.dram_tensor("cbias", [128, CPC], f32, kind="ExternalInput")
    sbias = nc.dram_tensor("sbias", [32, 1], f32, kind="ExternalInput")
    out = nc.dram_tensor("out", [128, NQ], f32, kind="ExternalOutput")

    with tile.TileContext(nc) as tc, ExitStack() as ctx:
        const = ctx.enter_context(tc.tile_pool(name="const", bufs=1))
        gpad_p = ctx.enter_context(tc.tile_pool(name="gpad", bufs=4))
        rel_p = ctx.enter_context(tc.tile_pool(name="relt", bufs=4))
        v_p = ctx.enter_context(tc.tile_pool(name="vt", bufs=4))
        gw_p = ctx.enter_context(tc.tile_pool(name="gw", bufs=2))
        y_p = ctx.enter_context(tc.tile_pool(name="ym", bufs=2))
        ys_p = ctx.enter_context(tc.tile_pool(name="ys", bufs=2))
        sc_p = ctx.enter_context(tc.tile_pool(name="scr", bufs=2))
        tp_p = ctx.enter_context(tc.tile_pool(name="tp", bufs=2, space="PSUM"))
        tsp_p = ctx.enter_context(tc.tile_pool(name="tsp", bufs=1, space="PSUM"))
        yp_p = ctx.enter_context(tc.tile_pool(name="yp", bufs=2, space="PSUM"))
        z_p = ctx.enter_context(tc.tile_pool(name="zp", bufs=2, space="PSUM"))

        ident = const.tile([128, 128], f32)
        make_identity(nc, ident[:])
        band_sb = const.tile([128, NSEG * JB], f32)
        nc.sync.dma_start(band_sb[:], band[:])
        bstub_sb = const.tile([10, 32], f32)
        nc.sync.dma_start(bstub_sb[:], bstub[:])
        pwt_sb = const.tile([JB, NSEG * D], f32)
        nc.sync.dma_start(pwt_sb[:], pwt[:])
        pstub_sb = const.tile([KSTUB, D], f32)
        nc.sync.dma_start(pstub_sb[:], pstub[:])
        cb_sb = const.tile([128, CPC], f32)
        nc.sync.dma_start(cb_sb[:], cbias[:])
        sb_sb = const.tile([32, 1], f32)
        nc.sync.dma_start(sb_sb[:], sbias[:])
        hI_sb = const.tile([128, NQ], i32)
        nc.sync.dma_start(hI_sb[:], hI[:])
        tI_sb = const.tile([128, NQ], i32)
        nc.sync.dma_start(tI_sb[:], tI[:])
        rI_sb = const.tile([128, NQ], i32)
        nc.sync.dma_start(rI_sb[:], rI[:])
        out_sb = const.tile([128, NQ], f32)

        nrep = 1
        if mode.startswith("x"):
            nrep = int(mode[1:])
            mode = "full"
        for _rep in range(nrep):
          for chunk in range(NCHUNK):
            gw = gw_p.tile([128, 5 * NB], f32)      # 4 main windows + stub
            ts_ps = tsp_p.tile([10, NB], f32)
            vts = []
            for btl in range(CHUNK_BT):
                q = chunk * CHUNK_BT + btl
                gpad = gpad_p.tile([128, D + 2], f32)
                if mode == "compute":
                    nc.vector.memset(gpad[:], 0.125)
                else:
                    nc.vector.memset(gpad[:, 0:1], 0.0)
                if mode != "compute":
                    nc.gpsimd.indirect_dma_start(
                        out=gpad[:, 1:D + 1], out_offset=None, in_=ent[:],
                        in_offset=bass.IndirectOffsetOnAxis(ap=hI_sb[:, q:q + 1], axis=0))
                    rt = rel_p.tile([128, D], f32)
                    nc.gpsimd.indirect_dma_start(
                        out=rt[:], out_offset=None, in_=rel[:],
                        in_offset=bass.IndirectOffsetOnAxis(ap=rI_sb[:, q:q + 1], axis=0))
                    nc.vector.tensor_copy(gpad[:, D + 1:D + 2], rt[:, 0:1])
                vt = v_p.tile([128, D], f32)
                if mode != "compute":
                    nc.gpsimd.indirect_dma_start(
                        out=vt[:], out_offset=None, in_=ent[:],
                        in_offset=bass.IndirectOffsetOnAxis(ap=tI_sb[:, q:q + 1], axis=0))
                else:
                    nc.vector.memset(vt[:], 0.25)
                vts.append(vt)
                if mode == "gather":
                    nc.vector.scalar_tensor_tensor(
                        out=sc_p.tile([128, D], f32, tag="scr", name="scrg")[:],
                        in0=vt[:], scalar=1.0, in1=gpad[:, 1:D + 1],
                        op0=Alu.mult, op1=Alu.mult,
                        accum_out=out_sb[:, q:q + 1])
                    continue

                tp = tp_p.tile([128, 512], f32)
                for s in range(4):
                    nc.tensor.transpose(tp[:, s * 128:(s + 1) * 128],
                                        gpad[:, JB * s:JB * s + 128], ident[:])
                nc.tensor.transpose(ts_ps[:, btl * 128:(btl + 1) * 128],
                                    gpad[:, 4 * JB:D + 2], ident[:])
                for s in range(4):
                    nc.vector.tensor_copy(
                        gw[:, s * NB + btl * 128:s * NB + (btl + 1) * 128],
                        tp[:, s * 128:(s + 1) * 128])
            if mode == "gather":
                continue
            nc.vector.tensor_copy(gw[0:10, 4 * NB:5 * NB], ts_ps[:])

            ym = y_p.tile([JB, NSEG * NB], f32)
            ystub = ys_p.tile([KSTUB, NB], f32)
            nc.vector.memset(ystub[32:33, :], 1.0)
            for c in range(CPC):
                for s in range(4):
                    cs = c * 4 + s
                    yp = yp_p.tile([JB, NB], f32)
                    nc.tensor.matmul(yp[:], band_sb[:, cs * JB:(cs + 1) * JB],
                                     gw[:, s * NB:(s + 1) * NB],
                                     start=True, stop=True)
                    if cs % 2 == 0 and mode != "dverelu":
                        nc.scalar.activation(
                            ym[:, cs * NB:(cs + 1) * NB], yp[:],
                            mybir.ActivationFunctionType.Relu,
                            bias=cb_sb[0:JB, c:c + 1])
                    else:
                        nc.vector.tensor_scalar(ym[:, cs * NB:(cs + 1) * NB],
                                                yp[:], cb_sb[0:JB, c:c + 1],
                                                0.0, Alu.add, Alu.max)
            yps = yp_p.tile([32, NB], f32, tag="yp")
            nc.tensor.matmul(yps[:], bstub_sb[:], gw[0:10, 4 * NB:5 * NB],
                             start=True, stop=True)
            nc.scalar.activation(ystub[0:32, :], yps[:],
                                 mybir.ActivationFunctionType.Relu,
                                 bias=sb_sb[:, 0:1])

            for btl in range(CHUNK_BT):
                q = chunk * CHUNK_BT + btl
                z = z_p.tile([128, D], f32)
                for i in range(NSEG):
                    nc.tensor.matmul(
                        z[:],
                        ym[:, i * NB + btl * 128:i * NB + (btl + 1) * 128],
                        pwt_sb[:, i * D:(i + 1) * D],
                        start=(i == 0), stop=False)
                nc.tensor.matmul(z[:],
                                 ystub[:, btl * 128:(btl + 1) * 128],
                                 pstub_sb[:], start=False, stop=True)
                scr = sc_p.tile([128, D], f32)
                nc.vector.scalar_tensor_tensor(
                    out=scr[:], in0=z[:], scalar=1.0, in1=vts[btl][:],
                    op0=Alu.mult, op1=Alu.mult,
                    accum_out=out_sb[:, q:q + 1])

        nc.sync.dma_start(out[:], out_sb[:])
    nc.finalize()
    return nc


def _host_prep(inputs):
    """Per-core input dicts from the full problem inputs."""
    ent = np.ascontiguousarray(np.asarray(inputs["ent"], dtype=np.float32))
    rel = np.ascontiguousarray(np.asarray(inputs["rel"], dtype=np.float32))
    w = np.asarray(inputs["conv_w"], dtype=np.float32)       # [32, 1, 3]
    cb = np.asarray(inputs["conv_b"], dtype=np.float32)      # [32]
    pw = np.asarray(inputs["proj_w"], dtype=np.float32)      # [512, 16384]
    pb = np.asarray(inputs["proj_b"], dtype=np.float32)      # [512]
    h = np.asarray(inputs["h"]).astype(np.int32)
    r = np.asarray(inputs["r"]).astype(np.int32)
    t = np.asarray(inputs["t"]).astype(np.int32)

    hI = np.ascontiguousarray(h.reshape(NQ, 128).T)
    rI = np.ascontiguousarray(r.reshape(NQ, 128).T)
    tI = np.ascontiguousarray(t.reshape(NQ, 128).T)

    jl = np.arange(JB)
    jl8 = np.arange(8)
    in_maps = []
    for m in range(NCORES):
        band = np.zeros((128, NSEG, JB), np.float32)
        bstub = np.zeros((10, 32), np.float32)
        pwt = np.zeros((JB, NSEG, D), np.float32)
        pstub = np.zeros((KSTUB, D), np.float32)
        cbias = np.zeros((128, CPC), np.float32)
        sbias = np.zeros((32, 1), np.float32)
        for c in range(CPC):
            cg = CPC * m + c
            cbias[:, c] = cb[cg]
            sbias[c * 8:(c + 1) * 8, 0] = cb[cg]
            for k in range(3):
                bstub[jl8 + k, c * 8 + jl8] = w[cg, 0, k]
            for s in range(4):
                cs = c * 4 + s
                for k in range(3):
                    band[jl + k, cs, jl] = w[cg, 0, k]
                pwt[:, cs, :] = pw[:, cg * D + JB * s: cg * D + JB * (s + 1)].T
            pstub[c * 8:(c + 1) * 8, :] = pw[:, cg * D + 504: cg * D + 512].T
        if m == 0:
            pstub[32] = pb
        in_maps.append({
            "ent": ent, "rel": rel, "hI": hI, "tI": tI, "rI": rI,
            "band": np.ascontiguousarray(band.reshape(128, NSEG * JB)),
            "bstub": bstub,
            "pwt": np.ascontiguousarray(pwt.reshape(JB, NSEG * D)),
            "pstub": pstub, "cbias": cbias, "sbias": sbias,
        })
    return in_maps


def _run(inputs, trace=False, tmpdir=None):
    from concourse.bass_utils import run_bass_kernel_spmd

    if "nc" not in _CACHE:
        _CACHE["nc"] = _build_nc()
    nc = _CACHE["nc"]
    in_maps = _host_prep(inputs)
    res = run_bass_kernel_spmd(nc, in_maps, core_ids=list(range(NCORES)),
                               trace=trace, tmpdir=tmpdir)
    total = np.zeros((128, NQ), np.float64)
    for mres in res.results:
        total += mres["out"].astype(np.float64)
    return total.T.reshape(B).astype(np.float32), res


def kernel(**inputs):
    out, _ = _run(inputs, trace=False)
    return out



# revision 2
# speedup vs baseline: 2.5386x; 2.5386x over previous
"""ConvTransE forward on 8 Trainium2 NeuronCores (Bass/Tile) — bf16 PE.

Math shortcut: the reference computes scores = x @ ent.T ([B, 100000]) and
returns scores[i, t[i]]; we only compute out[b] = x[b] . ent[t[b]].
The conv's retained slice [:, :512] depends only on ent[h] and rel[r][:, 0].

Sharding: tensor-parallel over the projection contraction dim (channels).
Core m owns channels [4m, 4m+4).  Every core:
  - gathers ent[h] rows (cast fp32->bf16 in the DMA), rel[r][:, 0] only,
    and ent[t] rows (fp32) via indirect DMA,
  - PE-transposes overlapping 128-wide windows of the gathered rows (bf16),
  - runs the conv as banded matmuls on the PE (bands built on host, bf16),
  - projects its K-slice:  z_m = relu(conv)_m @ proj_w_m^T  (K = 2048/core),
  - emits partial[b] = z_m[b] . ent[t[b]] via a fused multiply+row-sum (fp32).
proj_b rides along as an extra "ones" contraction row on core 0 only.
Host sums the 8 [2048] partials.
"""

import numpy as np

NE, NRR, D, C, B = 100000, 500, 512, 32, 2048
NCORES = 8
CPC = C // NCORES          # 4 channels per core
NQ = B // 128              # 16 batch tiles of 128
CHUNK_BT = 4               # batch tiles per pipeline chunk
NCHUNK = NQ // CHUNK_BT
NB = CHUNK_BT * 128        # 512 batch columns per chunk
JB = 126                   # conv j-block (126 outputs need a 128-wide input window)
NSEG = 16                  # (c, s) main contraction blocks per core
KSTUB = CPC * 8 + 1        # 33: packed j=504..511 stub rows + ones row

_CACHE = {}


def _build_nc():
    from contextlib import ExitStack

    import concourse.bass as bass
    import concourse.tile as tile
    from concourse import bacc, mybir
    from concourse.masks import make_identity

    f32 = mybir.dt.float32
    bf16 = mybir.dt.bfloat16
    i32 = mybir.dt.int32
    Alu = mybir.AluOpType

    nc = bacc.Bacc("TRN2", target_bir_lowering=False, debug=False,
                   num_devices=NCORES)

    ent = nc.dram_tensor("ent", [NE, D], f32, kind="ExternalInput")
    rel = nc.dram_tensor("rel", [NRR, D], f32, kind="ExternalInput")
    hI = nc.dram_tensor("hI", [128, NQ], i32, kind="ExternalInput")
    tI = nc.dram_tensor("tI", [128, NQ], i32, kind="ExternalInput")
    rI = nc.dram_tensor("rI", [128, NQ], i32, kind="ExternalInput")
    band = nc.dram_tensor("band", [128, NSEG * JB], bf16, kind="ExternalInput")
    bstub = nc.dram_tensor("bstub", [10, 32], bf16, kind="ExternalInput")
    pwt = nc.dram_tensor("pwt", [JB, NSEG * D], bf16, kind="ExternalInput")
    pstub = nc.dram_tensor("pstub", [KSTUB, D], bf16, kind="ExternalInput")
    cbias = nc.dram_tensor("cbias", [128, CPC], f32, kind="ExternalInput")
    sbias = nc.dram_tensor("sbias", [32, 1], f32, kind="ExternalInput")
    out = nc.dram_tensor("out", [128, NQ], f32, kind="ExternalOutput")

    with tile.TileContext(nc) as tc, ExitStack() as ctx:
        const = ctx.enter_context(tc.tile_pool(name="const", bufs=1))
        gpad_p = ctx.enter_context(tc.tile_pool(name="gpad", bufs=6))
        v_p = ctx.enter_context(tc.tile_pool(name="vt", bufs=6))
        gw_p = ctx.enter_context(tc.tile_pool(name="gw", bufs=2))
        y_p = ctx.enter_context(tc.tile_pool(name="ym", bufs=2))
        ys_p = ctx.enter_context(tc.tile_pool(name="ys", bufs=2))
        sc_p = ctx.enter_context(tc.tile_pool(name="scr", bufs=2))
        tp_p = ctx.enter_context(tc.tile_pool(name="tp", bufs=2, space="PSUM"))
        tsp_p = ctx.enter_context(tc.tile_pool(name="tsp", bufs=2, space="PSUM"))
        yp_p = ctx.enter_context(tc.tile_pool(name="yp", bufs=2, space="PSUM"))
        z_p = ctx.enter_context(tc.tile_pool(name="zp", bufs=2, space="PSUM"))

        ident = const.tile([128, 128], bf16)
        make_identity(nc, ident[:])
        band_sb = const.tile([128, NSEG * JB], bf16)
        nc.sync.dma_start(band_sb[:], band[:])
        bstub_sb = const.tile([10, 32], bf16)
        nc.sync.dma_start(bstub_sb[:], bstub[:])
        pwt_sb = const.tile([JB, NSEG * D], bf16)
        nc.sync.dma_start(pwt_sb[:], pwt[:])
        pstub_sb = const.tile([KSTUB, D], bf16)
        nc.sync.dma_start(pstub_sb[:], pstub[:])
        cb_sb = const.tile([128, CPC], f32)
        nc.sync.dma_start(cb_sb[:], cbias[:])
        sb_sb = const.tile([32, 1], f32)
        nc.sync.dma_start(sb_sb[:], sbias[:])
        hI_sb = const.tile([128, NQ], i32)
        nc.sync.dma_start(hI_sb[:], hI[:])
        tI_sb = const.tile([128, NQ], i32)
        nc.sync.dma_start(tI_sb[:], tI[:])
        rI_sb = const.tile([128, NQ], i32)
        nc.sync.dma_start(rI_sb[:], rI[:])
        out_sb = const.tile([128, NQ], f32)

        for chunk in range(NCHUNK):
            # gw columns: [btl*128 + j] per window s block of NB, i.e.
            # window-major as before: gw[:, s*NB + btl*128 + j]
            gw = gw_p.tile([128, 5 * NB], bf16)     # 4 main windows + stub
            ts_ps = tsp_p.tile([10, NB], bf16)
            vts = []
            for btl in range(CHUNK_BT):
                q = chunk * CHUNK_BT + btl
                gpad = gpad_p.tile([128, D + 2], bf16)
                nc.vector.memset(gpad[:, 0:1], 0.0)
                nc.gpsimd.indirect_dma_start(
                    out=gpad[:, 1:D + 1], out_offset=None, in_=ent[:],
                    in_offset=bass.IndirectOffsetOnAxis(ap=hI_sb[:, q:q + 1], axis=0))
                nc.gpsimd.indirect_dma_start(
                    out=gpad[:, D + 1:D + 2], out_offset=None, in_=rel[:, 0:1],
                    in_offset=bass.IndirectOffsetOnAxis(ap=rI_sb[:, q:q + 1], axis=0))
                vt = v_p.tile([128, D], f32)
                nc.gpsimd.indirect_dma_start(
                    out=vt[:], out_offset=None, in_=ent[:],
                    in_offset=bass.IndirectOffsetOnAxis(ap=tI_sb[:, q:q + 1], axis=0))
                vts.append(vt)

                tp = tp_p.tile([128, 512], bf16)
                for s in range(4):
                    nc.tensor.transpose(tp[:, s * 128:(s + 1) * 128],
                                        gpad[:, JB * s:JB * s + 128], ident[:])
                nc.tensor.transpose(ts_ps[:, btl * 128:(btl + 1) * 128],
                                    gpad[:, 4 * JB:D + 2], ident[:])
                for s in range(4):
                    nc.vector.tensor_copy(
                        gw[:, s * NB + btl * 128:s * NB + (btl + 1) * 128],
                        tp[:, s * 128:(s + 1) * 128])
            nc.vector.tensor_copy(gw[0:10, 4 * NB:5 * NB], ts_ps[:])

            ym = y_p.tile([JB, NSEG * NB], bf16)
            ystub = ys_p.tile([KSTUB, NB], bf16)
            nc.vector.memset(ystub[32:33, :], 1.0)
            for c in range(CPC):
                for s in range(4):
                    cs = c * 4 + s
                    yp = yp_p.tile([JB, NB], f32)
                    nc.tensor.matmul(yp[:], band_sb[:, cs * JB:(cs + 1) * JB],
                                     gw[:, s * NB:(s + 1) * NB],
                                     start=True, stop=True)
                    if cs % 2 == 0:
                        nc.scalar.activation(
                            ym[:, cs * NB:(cs + 1) * NB], yp[:],
                            mybir.ActivationFunctionType.Relu,
                            bias=cb_sb[0:JB, c:c + 1])
                    else:
                        nc.vector.tensor_scalar(ym[:, cs * NB:(cs + 1) * NB],
                                                yp[:], cb_sb[0:JB, c:c + 1],
                                                0.0, Alu.add, Alu.max)
            yps = yp_p.tile([32, NB], f32, tag="yp")
            nc.tensor.matmul(yps[:], bstub_sb[:], gw[0:10, 4 * NB:5 * NB],
                             start=True, stop=True)
            nc.scalar.activation(ystub[0:32, :], yps[:],
                                 mybir.ActivationFunctionType.Relu,
                                 bias=sb_sb[:, 0:1])

            for btl in range(CHUNK_BT):
                q = chunk * CHUNK_BT + btl
                z = z_p.tile([128, D], f32)
                for i in range(NSEG):
                    nc.tensor.matmul(
                        z[:],
                        ym[:, i * NB + btl * 128:i * NB + (btl + 1) * 128],
                        pwt_sb[:, i * D:(i + 1) * D],
                        start=(i == 0), stop=False)
                nc.tensor.matmul(z[:],
                                 ystub[:, btl * 128:(btl + 1) * 128],
                                 pstub_sb[:], start=False, stop=True)
                scr = sc_p.tile([128, D], f32)
                nc.vector.scalar_tensor_tensor(
                    out=scr[:], in0=z[:], scalar=1.0, in1=vts[btl][:],
                    op0=Alu.mult, op1=Alu.mult,
                    accum_out=out_sb[:, q:q + 1])

        nc.sync.dma_start(out[:], out_sb[:])
    nc.finalize()
    return nc


def _host_prep(inputs):
    """Per-core input dicts from the full problem inputs."""
    import ml_dtypes

    bf = ml_dtypes.bfloat16
    ent = np.ascontiguousarray(np.asarray(inputs["ent"], dtype=np.float32))
    rel = np.ascontiguousarray(np.asarray(inputs["rel"], dtype=np.float32))
    w = np.asarray(inputs["conv_w"], dtype=np.float32)       # [32, 1, 3]
    cb = np.asarray(inputs["conv_b"], dtype=np.float32)      # [32]
    pw = np.asarray(inputs["proj_w"], dtype=np.float32)      # [512, 16384]
    pb = np.asarray(inputs["proj_b"], dtype=np.float32)      # [512]
    h = np.asarray(inputs["h"]).astype(np.int32)
    r = np.asarray(inputs["r"]).astype(np.int32)
    t = np.asarray(inputs["t"]).astype(np.int32)

    hI = np.ascontiguousarray(h.reshape(NQ, 128).T)
    rI = np.ascontiguousarray(r.reshape(NQ, 128).T)
    tI = np.ascontiguousarray(t.reshape(NQ, 128).T)

    jl = np.arange(JB)
    jl8 = np.arange(8)
    in_maps = []
    for m in range(NCORES):
        band = np.zeros((128, NSEG, JB), np.float32)
        bstub = np.zeros((10, 32), np.float32)
        pwt = np.zeros((JB, NSEG, D), np.float32)
        pstub = np.zeros((KSTUB, D), np.float32)
        cbias = np.zeros((128, CPC), np.float32)
        sbias = np.zeros((32, 1), np.float32)
        for c in range(CPC):
            cg = CPC * m + c
            cbias[:, c] = cb[cg]
            sbias[c * 8:(c + 1) * 8, 0] = cb[cg]
            for k in range(3):
                bstub[jl8 + k, c * 8 + jl8] = w[cg, 0, k]
            for s in range(4):
                cs = c * 4 + s
                for k in range(3):
                    band[jl + k, cs, jl] = w[cg, 0, k]
                pwt[:, cs, :] = pw[:, cg * D + JB * s: cg * D + JB * (s + 1)].T
            pstub[c * 8:(c + 1) * 8, :] = pw[:, cg * D + 504: cg * D + 512].T
        if m == 0:
            pstub[32] = pb
        in_maps.append({
            "ent": ent, "rel": rel, "hI": hI, "tI": tI, "rI": rI,
            "band": np.ascontiguousarray(band.reshape(128, NSEG * JB)).astype(bf),
            "bstub": bstub.astype(bf),
            "pwt": np.ascontiguousarray(pwt.reshape(JB, NSEG * D)).astype(bf),
            "pstub": pstub.astype(bf), "cbias": cbias, "sbias": sbias,
        })
    return in_maps


def _run(inputs, trace=False, tmpdir=None):
    from concourse.bass_utils import run_bass_kernel_spmd

    if "nc" not in _CACHE:
        _CACHE["nc"] = _build_nc()
    nc = _CACHE["nc"]
    in_maps = _host_prep(inputs)
    res = run_bass_kernel_spmd(nc, in_maps, core_ids=list(range(NCORES)),
                               trace=trace, tmpdir=tmpdir)
    total = np.zeros((128, NQ), np.float64)
    for mres in res.results:
        total += mres["out"].astype(np.float64)
    return total.T.reshape(B).astype(np.float32), res


def kernel(**inputs):
    out, _ = _run(inputs, trace=False)
    return out


# revision 3
# speedup vs baseline: 2.7763x; 1.0936x over previous
"""ConvTransE forward on 8 Trainium2 NeuronCores (Bass/Tile) — bf16 PE, v3.

Math shortcut: the reference computes scores = x @ ent.T ([B, 100000]) and
returns scores[i, t[i]]; we only compute out[b] = x[b] . ent[t[b]].
The conv's retained slice [:, :512] depends only on ent[h] and rel[r][:, 0].

Sharding: tensor-parallel over the projection contraction dim (channels).
Core m owns channels [4m, 4m+4).  Every core:
  - gathers ent[h] rows (cast fp32->bf16 in the DMA), rel[r][:, 0] only,
    and ent[t] rows (fp32) via indirect DMA,
  - PE-transposes overlapping 128-wide windows of the gathered rows (bf16),
  - runs the conv as banded matmuls on the PE (bands built on host, bf16),
  - projects its K-slice:  z_m = relu(conv)_m @ proj_w_m^T  (K = 2048/core),
  - emits partial[b] = z_m[b] . ent[t[b]] via a fused multiply+row-sum (fp32).
proj_b rides along as an extra "ones" contraction row on core 0 only.
Host sums the 8 [2048] partials.

v3: index DMAs issued before weight DMAs (gathers start ~13us earlier),
h/rel gathers issued before t gathers inside a chunk, per-btl transposes
packed into one [128, 640] PSUM tile copied with a single DVE op into a
btl-major gw, conv/stub matmuls read gw through strided 3D APs.
"""

import numpy as np

NE, NRR, D, C, B = 100000, 500, 512, 32, 2048
NCORES = 8
CPC = C // NCORES          # 4 channels per core
NQ = B // 128              # 16 batch tiles of 128
CHUNK_BT = 4               # batch tiles per pipeline chunk
NCHUNK = NQ // CHUNK_BT
NB = CHUNK_BT * 128        # 512 batch columns per chunk
JB = 126                   # conv j-block (126 outputs need a 128-wide input window)
NSEG = 16                  # (c, s) main contraction blocks per core
KSTUB = CPC * 8 + 1        # 33: packed j=504..511 stub rows + ones row
GWW = 640                  # per-btl gw block: 4 windows * 128 + stub 128

_CACHE = {}


def _build_nc():
    from contextlib import ExitStack

    import concourse.bass as bass
    import concourse.tile as tile
    from concourse import bacc, mybir
    from concourse.masks import make_identity

    f32 = mybir.dt.float32
    bf16 = mybir.dt.bfloat16
    i32 = mybir.dt.int32
    Alu = mybir.AluOpType

    nc = bacc.Bacc("TRN2", target_bir_lowering=False, debug=False,
                   num_devices=NCORES)

    ent = nc.dram_tensor("ent", [NE, D], f32, kind="ExternalInput")
    rel = nc.dram_tensor("rel", [NRR, D], f32, kind="ExternalInput")
    hI = nc.dram_tensor("hI", [128, NQ], i32, kind="ExternalInput")
    tI = nc.dram_tensor("tI", [128, NQ], i32, kind="ExternalInput")
    rI = nc.dram_tensor("rI", [128, NQ], i32, kind="ExternalInput")
    band = nc.dram_tensor("band", [128, NSEG * JB], bf16, kind="ExternalInput")
    bstub = nc.dram_tensor("bstub", [10, 32], bf16, kind="ExternalInput")
    pwt = nc.dram_tensor("pwt", [JB, NSEG * D], bf16, kind="ExternalInput")
    pstub = nc.dram_tensor("pstub", [KSTUB, D], bf16, kind="ExternalInput")
    cbias = nc.dram_tensor("cbias", [128, CPC], f32, kind="ExternalInput")
    sbias = nc.dram_tensor("sbias", [32, 1], f32, kind="ExternalInput")
    out = nc.dram_tensor("out", [128, NQ], f32, kind="ExternalOutput")

    with tile.TileContext(nc) as tc, ExitStack() as ctx:
        const = ctx.enter_context(tc.tile_pool(name="const", bufs=1))
        gpad_p = ctx.enter_context(tc.tile_pool(name="gpad", bufs=6))
        v_p = ctx.enter_context(tc.tile_pool(name="vt", bufs=8))
        gw_p = ctx.enter_context(tc.tile_pool(name="gw", bufs=2))
        y_p = ctx.enter_context(tc.tile_pool(name="ym", bufs=2))
        ys_p = ctx.enter_context(tc.tile_pool(name="ys", bufs=2))
        sc_p = ctx.enter_context(tc.tile_pool(name="scr", bufs=2))
        tp_p = ctx.enter_context(tc.tile_pool(name="tp", bufs=3, space="PSUM"))
        yp_p = ctx.enter_context(tc.tile_pool(name="yp", bufs=2, space="PSUM"))
        z_p = ctx.enter_context(tc.tile_pool(name="zp", bufs=2, space="PSUM"))

        # tiny index tables first: the first gathers depend only on these
        hI_sb = const.tile([128, NQ], i32)
        nc.sync.dma_start(hI_sb[:], hI[:])
        rI_sb = const.tile([128, NQ], i32)
        nc.sync.dma_start(rI_sb[:], rI[:])
        tI_sb = const.tile([128, NQ], i32)
        nc.sync.dma_start(tI_sb[:], tI[:])
        ident = const.tile([128, 128], bf16)
        make_identity(nc, ident[:])
        band_sb = const.tile([128, NSEG * JB], bf16)
        nc.sync.dma_start(band_sb[:], band[:])
        bstub_sb = const.tile([10, 32], bf16)
        nc.sync.dma_start(bstub_sb[:], bstub[:])
        cb_sb = const.tile([128, CPC], f32)
        nc.sync.dma_start(cb_sb[:], cbias[:])
        sb_sb = const.tile([32, 1], f32)
        nc.sync.dma_start(sb_sb[:], sbias[:])
        pstub_sb = const.tile([KSTUB, D], bf16)
        nc.sync.dma_start(pstub_sb[:], pstub[:])
        # largest last: only needed once the first projection starts
        pwt_sb = const.tile([JB, NSEG * D], bf16)
        nc.sync.dma_start(pwt_sb[:], pwt[:])
        out_sb = const.tile([128, NQ], f32)

        for chunk in range(NCHUNK):
            # gw: per-btl blocks of 640 = [4 windows x 128 | stub 128]
            gw = gw_p.tile([128, CHUNK_BT * GWW], bf16)
            gwv = gw[:].rearrange("p (b w c) -> p b w c", b=CHUNK_BT, w=5, c=128)
            gpads = []
            for btl in range(CHUNK_BT):
                q = chunk * CHUNK_BT + btl
                gpad = gpad_p.tile([128, D + 2], bf16)
                nc.vector.memset(gpad[:, 0:1], 0.0)
                nc.gpsimd.indirect_dma_start(
                    out=gpad[:, 1:D + 1], out_offset=None, in_=ent[:],
                    in_offset=bass.IndirectOffsetOnAxis(ap=hI_sb[:, q:q + 1], axis=0))
                nc.gpsimd.indirect_dma_start(
                    out=gpad[:, D + 1:D + 2], out_offset=None, in_=rel[:, 0:1],
                    in_offset=bass.IndirectOffsetOnAxis(ap=rI_sb[:, q:q + 1], axis=0))
                gpads.append(gpad)
            vts = []
            for btl in range(CHUNK_BT):
                q = chunk * CHUNK_BT + btl
                vt = v_p.tile([128, D], f32)
                nc.gpsimd.indirect_dma_start(
                    out=vt[:], out_offset=None, in_=ent[:],
                    in_offset=bass.IndirectOffsetOnAxis(ap=tI_sb[:, q:q + 1], axis=0))
                vts.append(vt)
            for btl in range(CHUNK_BT):
                gpad = gpads[btl]
                tp = tp_p.tile([128, GWW], bf16)
                for s in range(4):
                    nc.tensor.transpose(tp[:, s * 128:(s + 1) * 128],
                                        gpad[:, JB * s:JB * s + 128], ident[:])
                nc.tensor.transpose(tp[0:10, 512:640],
                                    gpad[:, 4 * JB:D + 2], ident[:])
                nc.vector.tensor_copy(gw[:, btl * GWW:(btl + 1) * GWW], tp[:])

            ym = y_p.tile([JB, NSEG * NB], bf16)
            ystub = ys_p.tile([KSTUB, NB], bf16)
            nc.vector.memset(ystub[32:33, :], 1.0)
            for c in range(CPC):
                for s in range(4):
                    cs = c * 4 + s
                    yp = yp_p.tile([JB, NB], f32)
                    nc.tensor.matmul(yp[:], band_sb[:, cs * JB:(cs + 1) * JB],
                                     gwv[:, :, s, :],
                                     start=True, stop=True)
                    if cs % 2 == 0:
                        nc.scalar.activation(
                            ym[:, cs * NB:(cs + 1) * NB], yp[:],
                            mybir.ActivationFunctionType.Relu,
                            bias=cb_sb[0:JB, c:c + 1])
                    else:
                        nc.vector.tensor_scalar(ym[:, cs * NB:(cs + 1) * NB],
                                                yp[:], cb_sb[0:JB, c:c + 1],
                                                0.0, Alu.add, Alu.max)
            yps = yp_p.tile([32, NB], f32, tag="yp")
            nc.tensor.matmul(yps[:], bstub_sb[:], gwv[0:10, :, 4, :],
                             start=True, stop=True)
            nc.scalar.activation(ystub[0:32, :], yps[:],
                                 mybir.ActivationFunctionType.Relu,
                                 bias=sb_sb[:, 0:1])

            for btl in range(CHUNK_BT):
                q = chunk * CHUNK_BT + btl
                z = z_p.tile([128, D], f32)
                for i in range(NSEG):
                    nc.tensor.matmul(
                        z[:],
                        ym[:, i * NB + btl * 128:i * NB + (btl + 1) * 128],
                        pwt_sb[:, i * D:(i + 1) * D],
                        start=(i == 0), stop=False)
                nc.tensor.matmul(z[:],
                                 ystub[:, btl * 128:(btl + 1) * 128],
                                 pstub_sb[:], start=False, stop=True)
                scr = sc_p.tile([128, D], f32)
                nc.vector.scalar_tensor_tensor(
                    out=scr[:], in0=z[:], scalar=1.0, in1=vts[btl][:],
                    op0=Alu.mult, op1=Alu.mult,
                    accum_out=out_sb[:, q:q + 1])

        nc.sync.dma_start(out[:], out_sb[:])
    nc.finalize()
    return nc


def _host_prep(inputs):
    """Per-core input dicts from the full problem inputs."""
    import ml_dtypes

    bf = ml_dtypes.bfloat16
    ent = np.ascontiguousarray(np.asarray(inputs["ent"], dtype=np.float32))
    rel = np.ascontiguousarray(np.asarray(inputs["rel"], dtype=np.float32))
    w = np.asarray(inputs["conv_w"], dtype=np.float32)       # [32, 1, 3]
    cb = np.asarray(inputs["conv_b"], dtype=np.float32)      # [32]
    pw = np.asarray(inputs["proj_w"], dtype=np.float32)      # [512, 16384]
    pb = np.asarray(inputs["proj_b"], dtype=np.float32)      # [512]
    h = np.asarray(inputs["h"]).astype(np.int32)
    r = np.asarray(inputs["r"]).astype(np.int32)
    t = np.asarray(inputs["t"]).astype(np.int32)

    hI = np.ascontiguousarray(h.reshape(NQ, 128).T)
    rI = np.ascontiguousarray(r.reshape(NQ, 128).T)
    tI = np.ascontiguousarray(t.reshape(NQ, 128).T)

    jl = np.arange(JB)
    jl8 = np.arange(8)
    in_maps = []
    for m in range(NCORES):
        band = np.zeros((128, NSEG, JB), np.float32)
        bstub = np.zeros((10, 32), np.float32)
        pwt = np.zeros((JB, NSEG, D), np.float32)
        pstub = np.zeros((KSTUB, D), np.float32)
        cbias = np.zeros((128, CPC), np.float32)
        sbias = np.zeros((32, 1), np.float32)
        for c in range(CPC):
            cg = CPC * m + c
            cbias[:, c] = cb[cg]
            sbias[c * 8:(c + 1) * 8, 0] = cb[cg]
            for k in range(3):
                bstub[jl8 + k, c * 8 + jl8] = w[cg, 0, k]
            for s in range(4):
                cs = c * 4 + s
                for k in range(3):
                    band[jl + k, cs, jl] = w[cg, 0, k]
                pwt[:, cs, :] = pw[:, cg * D + JB * s: cg * D + JB * (s + 1)].T
            pstub[c * 8:(c + 1) * 8, :] = pw[:, cg * D + 504: cg * D + 512].T
        if m == 0:
            pstub[32] = pb
        in_maps.append({
            "ent": ent, "rel": rel, "hI": hI, "tI": tI, "rI": rI,
            "band": np.ascontiguousarray(band.reshape(128, NSEG * JB)).astype(bf),
            "bstub": bstub.astype(bf),
            "pwt": np.ascontiguousarray(pwt.reshape(JB, NSEG * D)).astype(bf),
            "pstub": pstub.astype(bf), "cbias": cbias, "sbias": sbias,
        })
    return in_maps


def _run(inputs, trace=False, tmpdir=None):
    from concourse.bass_utils import run_bass_kernel_spmd

    if "nc" not in _CACHE:
        _CACHE["nc"] = _build_nc()
    nc = _CACHE["nc"]
    in_maps = _host_prep(inputs)
    res = run_bass_kernel_spmd(nc, in_maps, core_ids=list(range(NCORES)),
                               trace=trace, tmpdir=tmpdir)
    total = np.zeros((128, NQ), np.float64)
    for mres in res.results:
        total += mres["out"].astype(np.float64)
    return total.T.reshape(B).astype(np.float32), res


def kernel(**inputs):
    out, _ = _run(inputs, trace=False)
    return out
